# revision 7
# baseline (speedup 1.0000x reference)
"""Trainium2 Bass kernel for nn_AttentionLayer (segment softmax attention pooling).

Computation (reference):
    h = tanh(x @ W1 + b1)            # [N, A]
    s = h @ W2 + b2                  # [N, 1]
    per-segment softmax over s, out[b] = sum_i softmax_w_i * x_i   # [B, D]

v2 strategy (per core, N/8 = 62500 rows, all device streams in bf16):
  - Host pre-transposes x to xT [D, rows] and casts to bf16 (tolerance is
    2e-2; bf16 costs ~0.4%).  DMA halves vs f32.
  - PE pass 1: hraw = W1^T @ xT per 1536-row chunk (W1 stationary).
  - ACT: th = tanh(hraw + b1) -> SBUF bf16.  This is the hard floor:
    8M lane-elements = ~64us at 1.2 GHz.  Everything else hides under it.
  - PE pass 2: scores via 12 sub-matmuls per chunk (W2 [A,1] stationary,
    rhs = th 128-column slices), each writing a [1,128] PSUM row into a
    compact score-batch tile [60,128] (5 chunks/batch).  Scores land on
    partitions instead of along the free dim, so exp is ~100x cheaper
    than the baseline's broadcast exp (ACT cost scales with free size).
  - ACT: e = exp(s + b2) on the compact batch (bf16 out).
  - Tiny SP-DMA unpacks each chunk's 12 rows into a contiguous e_row
    [1, 1536], exported to HBM for the host and fed to the multiplier.
  - DVE: affine_mul_reduce(xT_chunk * e_bcast) -> per-chunk window sum
    wacc[:, c].  e_bcast is either a stride-0 partition-broadcast AP read
    of e_row (BCAST_MODE="ap") or a GpSimd partition_broadcast into a
    [128, 1536] tile (BCAST_MODE="gpsimd").
  - Segment logic on host, as in the baseline: windows fully inside one
    segment use the device sums; windows containing a boundary are
    recomputed on the host from x and the exported e.  Denominators via
    bincount over e.  exp without max-subtraction is safe (|s| < ~4) and
    numerator/denominator use identical e values.
"""

import numpy as np
import ml_dtypes

# Problem constants (hardcoded per contract; kernel.py must be self-contained).
N = 500_000
D = 128
A = 128
B = 256
NCORES = 8
RPC = N // NCORES            # rows per core = 62500
CHUNK = 1536                 # rows per streamed tile (3 PSUM banks in f32)
SUBW = 512                   # score sub-matmul moving width
SUBJ = CHUNK // SUBW         # score sub-matmuls per chunk = 3
NCHUNK = -(-RPC // CHUNK)    # 41
RPAD = NCHUNK * CHUNK        # 62976
REGC = 10                    # chunks per score region (30 of 32 PSUM rows)
REG_ROWS = REGC * SUBJ       # 30
NREG = -(-NCHUNK // REGC)    # 5 (last region holds 1 chunk)
EROWS = NCHUNK * SUBJ        # valid eout rows = 123
MM_N = 512                   # h-matmul moving slice

BCAST_MODE = "gpsimd"        # "ap" (stride-0 partition read) is rejected at lowering

_prog_cache = {}


def _build_program(b2val: float, mode: str):
    import concourse.bacc as bacc
    from concourse import mybir
    from concourse.tile import TileContext

    f32 = mybir.dt.float32
    bf16 = mybir.dt.bfloat16
    nc = bacc.Bacc("TRN2", target_bir_lowering=False, debug=False,
                   num_devices=NCORES)

    xt = nc.dram_tensor("xt", [D, RPAD], bf16, kind="ExternalInput")
    w1 = nc.dram_tensor("w1", [D, A], bf16, kind="ExternalInput")
    w2oh = nc.dram_tensor("w2oh", [A, REG_ROWS * 32], bf16,
                          kind="ExternalInput")
    b1 = nc.dram_tensor("b1", [A, 1], f32, kind="ExternalInput")
    b2 = nc.dram_tensor("b2", [128, 1], f32, kind="ExternalInput")
    wacc = nc.dram_tensor("wacc", [D, NCHUNK], f32, kind="ExternalOutput")
    eout = nc.dram_tensor("eout", [NREG * REG_ROWS, SUBW], bf16,
                          kind="ExternalOutput")

    with TileContext(nc) as tc:
        with tc.tile_pool(name="const", bufs=1) as cpool, \
             tc.tile_pool(name="xtp", bufs=14) as xpool, \
             tc.tile_pool(name="thp", bufs=3) as thpool, \
             tc.tile_pool(name="ebp", bufs=2) as ebpool, \
             tc.tile_pool(name="erp", bufs=2) as erpool, \
             tc.tile_pool(name="ebcp", bufs=3) as ebcpool, \
             tc.tile_pool(name="junkp", bufs=3) as jpool, \
             tc.tile_pool(name="accp", bufs=1) as apool, \
             tc.tile_pool(name="psh", bufs=2, space="PSUM") as psh, \
             tc.tile_pool(name="pss", bufs=2, space="PSUM") as pss:

            w1sb = cpool.tile([D, A], bf16, tag="w1")
            w2sb = cpool.tile([A, REG_ROWS * 32], bf16, tag="w2")
            b1sb = cpool.tile([A, 1], f32, tag="b1")
            b2sb = cpool.tile([128, 1], f32, tag="b2")
            nc.sync.dma_start(out=w1sb[:], in_=w1[:])
            nc.sync.dma_start(out=w2sb[:], in_=w2oh[:])
            nc.sync.dma_start(out=b1sb[:], in_=b1[:])
            nc.sync.dma_start(out=b2sb[:], in_=b2[:])

            waccsb = apool.tile([D, NCHUNK], f32, tag="wacc")

            xtiles, ths, hregs = {}, {}, {}
            sbatches, ebatches, erows = {}, {}, {}

            def load(c):
                if c >= NCHUNK:
                    return
                xtile = xpool.tile([D, CHUNK], bf16, tag="x")
                nc.sync.dma_start(out=xtile[:],
                                  in_=xt[:, c * CHUNK:(c + 1) * CHUNK])
                xtiles[c] = xtile

            def hmm(c):
                hreg = psh.tile([128, CHUNK], f32, tag="hreg")
                hregs[c] = hreg
                xtile = xtiles[c]
                for i in range(CHUNK // MM_N):
                    nc.tensor.matmul(
                        out=hreg[:, i * MM_N:(i + 1) * MM_N],
                        lhsT=w1sb[:],
                        rhs=xtile[:, i * MM_N:(i + 1) * MM_N],
                        start=True, stop=True)

            def tanh(c):
                th = thpool.tile([A, CHUNK], bf16, tag="th")
                ths[c] = th
                nc.scalar.activation(
                    out=th[:], in_=hregs.pop(c),
                    func=mybir.ActivationFunctionType.Tanh,
                    bias=b1sb[:, 0:1])

            def submms(c):
                g, cc = c // REGC, c % REGC
                if cc == 0:
                    sbatches[g] = pss.tile([32, SUBW], f32, tag="sb",
                                           name="sb")
                sb = sbatches[g]
                th = ths.pop(c)
                last_c = min((g + 1) * REGC, NCHUNK) - 1
                for j in range(SUBJ):
                    r = cc * SUBJ + j
                    # one-hot stationary drops the scores on region row r;
                    # the whole [32, SUBW] region accumulates across the
                    # region's sub-matmuls (start clears it on the first).
                    nc.tensor.matmul(out=sb[0:32, :],
                                     lhsT=w2sb[:, r * 32:(r + 1) * 32],
                                     rhs=th[:, j * SUBW:(j + 1) * SUBW],
                                     start=(r == 0),
                                     stop=(c == last_c and j == SUBJ - 1),
                                     skip_group_check=True)

            def region_tail(g):
                lo = g * REGC
                hi = min(lo + REGC, NCHUNK)
                rows = (hi - lo) * SUBJ
                eb = ebpool.tile([REG_ROWS, SUBW], bf16, tag="eb")
                ebatches[g] = eb
                nc.scalar.activation(
                    out=eb[0:rows, :], in_=sbatches.pop(g)[0:rows, :],
                    func=mybir.ActivationFunctionType.Exp,
                    bias=b2sb[0:rows, 0:1])
                nc.sync.dma_start(out=eout[g * REG_ROWS:g * REG_ROWS + rows, :],
                                  in_=eb[0:rows, :])
                er = erpool.tile([1, REGC * CHUNK], bf16, tag="er",
                                 name="er")
                nc.sync.dma_start(out=er[0:1, 0:rows * SUBW],
                                  in_=eb[0:rows, :])
                for c in range(lo, hi):
                    cc = c - lo
                    xtile = xtiles.pop(c)
                    ebc = ebcpool.tile([128, CHUNK], bf16, tag="ebc")
                    nc.gpsimd.partition_broadcast(
                        ebc[:], er[0:1, cc * CHUNK:(cc + 1) * CHUNK])
                    # fused multiply + free-dim sum; a real packed bf16
                    # product tile keeps the op eligible for DVE 2x modes.
                    prod = jpool.tile([D, CHUNK], bf16, tag="prod")
                    nc.vector.scalar_tensor_tensor(
                        out=prod[:],
                        in0=xtile[:],
                        scalar=1.0,
                        in1=ebc[:],
                        op0=mybir.AluOpType.mult,
                        op1=mybir.AluOpType.mult,
                        accum_out=waccsb[:, c:c + 1])

            LOOK = 8
            for c in range(min(LOOK, NCHUNK)):
                load(c)
            hmm(0)
            for c in range(NCHUNK):
                load(c + LOOK)
                if c + 1 < NCHUNK:
                    hmm(c + 1)
                tanh(c)
                submms(c)
                # region g's tail is emitted one chunk after the region
                # closes so the trailing exp never sits at the ACT queue
                # head in front of the next tanh.
                if c >= 1 and ((c - 1) % REGC == REGC - 1):
                    region_tail((c - 1) // REGC)
            region_tail(NREG - 1)

            nc.sync.dma_start(out=wacc[:], in_=waccsb[:])

    nc.compile()
    return nc


def _run_device(xt_shards, W1, W2, b1, b2, mode, trace=False):
    from concourse.bass_utils import run_bass_kernel_spmd

    key = (float(b2), mode)
    if key not in _prog_cache:
        _prog_cache[key] = _build_program(float(b2), mode)
    nc = _prog_cache[key]

    bf16 = ml_dtypes.bfloat16
    w1_in = np.ascontiguousarray(W1.astype(bf16))
    w2oh = np.zeros((A, REG_ROWS, 32), dtype=np.float32)
    for r in range(REG_ROWS):
        w2oh[:, r, r % 32] = W2.reshape(-1)
    w2_in = np.ascontiguousarray(w2oh.reshape(A, REG_ROWS * 32).astype(bf16))
    b1_in = np.ascontiguousarray(b1.reshape(A, 1), dtype=np.float32)
    b2_in = np.full((128, 1), np.float32(b2), dtype=np.float32)

    in_maps = [{"xt": xt_shards[i], "w1": w1_in, "w2oh": w2_in, "b1": b1_in,
                "b2": b2_in}
               for i in range(NCORES)]
    res = run_bass_kernel_spmd(nc, in_maps, core_ids=list(range(NCORES)),
                               trace=trace)
    return res


def kernel(x, batch_index, W1, b1, W2, b2, _want_results=False, _trace=False):
    x = np.ascontiguousarray(np.asarray(x, dtype=np.float32))
    bi64 = np.asarray(batch_index).astype(np.int64)
    W1 = np.asarray(W1, dtype=np.float32)
    b1 = np.asarray(b1, dtype=np.float32)
    W2 = np.asarray(W2, dtype=np.float32)
    b2v = float(np.asarray(b2, dtype=np.float32).reshape(-1)[0])

    assert x.shape == (N, D)

    # Host pre-transpose + bf16 cast: xT [D, N], then zero-padded shards.
    bf16 = ml_dtypes.bfloat16
    xtf = np.ascontiguousarray(x.T.astype(bf16))
    xt_shards = []
    for i in range(NCORES):
        sh = np.zeros((D, RPAD), dtype=bf16)
        sh[:, :RPC] = xtf[:, i * RPC:(i + 1) * RPC]
        xt_shards.append(sh)

    res = _run_device(xt_shards, W1, W2, b1, b2v, BCAST_MODE, trace=_trace)

    # Gather device outputs.  eout rows are (chunk, sub)-major so a plain
    # reshape recovers instance order.
    e = np.empty(N, dtype=np.float32)
    waccs = []
    for i in range(NCORES):
        eo = np.asarray(res.results[i]["eout"])[:EROWS]
        e[i * RPC:(i + 1) * RPC] = \
            eo.astype(np.float32).reshape(-1)[:RPC]
        waccs.append(np.asarray(res.results[i]["wacc"]))

    # Denominators: segment sums of e (same values the device used).
    denom = np.bincount(bi64, weights=e.astype(np.float64), minlength=B)

    # Numerators: pure windows from device sums; boundary windows recomputed.
    WIN = CHUNK
    num = np.zeros((B, D), dtype=np.float64)
    for i in range(NCORES):
        wacc_i = waccs[i]
        base = i * RPC
        for w in range(NCHUNK):
            glo = base + w * WIN
            if glo >= base + RPC:
                break
            ghi = min(glo + WIN, base + RPC)
            b_first = bi64[glo]
            b_last = bi64[ghi - 1]
            if b_first == b_last:
                # Window entirely in one segment (zero-pad rows contribute 0).
                num[b_first] += wacc_i[:, w]
            else:
                sub = bi64[glo:ghi]
                cuts = np.flatnonzero(np.diff(sub)) + 1
                bounds = np.concatenate(([0], cuts, [ghi - glo]))
                for k in range(len(bounds) - 1):
                    lo, hi = glo + bounds[k], glo + bounds[k + 1]
                    num[sub[bounds[k]]] += \
                        e[lo:hi].astype(np.float64) @ x[lo:hi].astype(np.float64)

    dn = denom[:, None]
    out = np.divide(num, dn, out=np.zeros_like(num), where=dn > 0)
    out = out.astype(np.float32)
    if _want_results:
        return out, res
    return out
